# revision 3
# baseline (speedup 1.0000x reference)
"""GAT (3-layer, PyG-style) on 8 Trainium2 NeuronCores via Bass/Tile.

Strategy: shard destination nodes (and their incident edges) across the 8
cores. Per layer: sharded dense matmul h = x @ W on PE; AllGather of packed
node rows ([fp8 h | bf16 a_src]) to every core's DRAM; per-dst-tile row
gathers (dma_gather); edge softmax + weighted aggregation expressed as
128-edge-chunk matmuls against 0/1 selection matrices (stored fp8, host
prepared); post-aggregation normalization by the segment-sum reciprocal;
ELU between layers; log_softmax at the end.

v2 (vs baseline): fp8 h payload in the gathered rows (768B/edge instead of
1280B), fp8 selection matrices as matmul lhsT (halves R traffic), attention
dots computed from PSUM, single rs/rt DMA per tile, deeper gather buffering.
"""

import os
import sys
import functools

import numpy as np

for _p in ("/root/.axon_site/_ro/trn_rl_repo", "/opt/trn_rl_repo"):
    if os.path.isdir(_p) and _p not in sys.path:
        sys.path.insert(0, _p)

import ml_dtypes

import concourse.bass as bass
import concourse.bacc as bacc
import concourse.mybir as mybir
import concourse.tile as tile
from concourse import bass_utils

BF16 = mybir.dt.bfloat16
FP8 = mybir.dt.float8e4
F32 = mybir.dt.float32
I16 = mybir.dt.int16
AF = mybir.ActivationFunctionType
OP = mybir.AluOpType

NEG_SLOPE = 0.2
N_CORES = 8


class Cfg:
    def __init__(self, n=20000, e=320000, in_dim=512, hid=64, heads=8, out_dim=64,
                 cpt=16, has_bias=True):
        self.has_bias = has_bias
        self.n, self.e = n, e
        self.in_dim, self.hid, self.heads, self.out_dim = in_dim, hid, heads, out_dim
        self.kc = in_dim // 128          # K chunks for dense matmuls
        self.cpt = cpt                   # chunks (of 128 edges) per dst tile
        # filled by prep:
        self.tpc = None                  # tiles per core
        self.nslot = None                # dst slots per core (tpc*128)


# ----------------------------------------------------------------- host prep

def _pack_tiles(counts_nonself, n, cpt):
    """Pack consecutive (sorted) dst nodes into tiles of <=128 nodes and
    <= (cpt-1)*128 non-self edges (chunk 0 is reserved for the self loops).
    Returns list of (node_start, node_count)."""
    emax = (cpt - 1) * 128
    tiles = []
    ns = 0
    while ns < n:
        nc_ = 0
        ec = 0
        while ns + nc_ < n and nc_ < 128 and ec + counts_nonself[ns + nc_] <= emax:
            ec += counts_nonself[ns + nc_]
            nc_ += 1
        assert nc_ > 0, "single node exceeds tile edge budget"
        tiles.append((ns, nc_))
        ns += nc_
    return tiles


def prep(cfg, edge_index):
    """All graph-static metadata. Returns dict of per-core numpy arrays.

    Edge slot layout per tile: chunk 0 holds the self loops (edge slot
    (0, p) = self loop of dst-local node p; loaded by direct DMA from
    h_own, not gathered), chunks 1..cpt-1 hold the real edges sorted by
    dst. hidx values are in "AllGather-split" slot space: the AllGather is
    issued in two halves, so slot s of core c lives at c*H1+s (s < H1) or
    NC*H1 + c*H2 + (s-H1) in the gathered table.
    """
    n, e, cpt = cfg.n, cfg.e, cfg.cpt
    src_r = edge_index[0].astype(np.int64)
    dst_r = edge_index[1].astype(np.int64)
    order = np.argsort(dst_r, kind="stable")
    src_s, dst_s = src_r[order], dst_r[order]

    counts = np.bincount(dst_s, minlength=n)
    tiles = _pack_tiles(counts, n, cpt)
    tpc = (len(tiles) + N_CORES - 1) // N_CORES
    nslot = tpc * 128
    cfg.tpc, cfg.nslot = tpc, nslot
    cfg.h1 = (tpc // 2) * 128            # first AllGather half (rows)
    cfg.h2 = nslot - cfg.h1
    while len(tiles) < tpc * N_CORES:
        tiles.append((n, 0))  # empty tiles

    # node -> (padded-global slot)
    pg = np.full(n, -1, np.int64)
    node_of_slot = np.full(N_CORES * nslot, -1, np.int64)
    for t, (ns, cnt) in enumerate(tiles):
        core, tl = divmod(t, tpc)
        s0 = core * nslot + tl * 128
        pg[ns:ns + cnt] = s0 + np.arange(cnt)
        node_of_slot[s0:s0 + cnt] = np.arange(ns, ns + cnt)

    def agrow(slot):
        return slot

    edge_ptr = np.searchsorted(dst_s, np.arange(n + 1))

    ecap = cpt * 128
    enon = (cpt - 1) * 128               # non-self edge capacity per tile
    S = tpc * cpt  # chunk slots per core per layer
    hidx = np.zeros((N_CORES, S * 128), np.int16)      # src row per edge slot
    hidx_orig = np.zeros((N_CORES, S * 128), np.int64)  # original slot space
    dstloc = np.full((N_CORES, S * 128), -1.0, np.float32)
    waste_num = 0
    for t, (ns, cnt) in enumerate(tiles):
        if cnt == 0:
            continue
        core, tl = divmod(t, tpc)
        e0, e1 = edge_ptr[ns], edge_ptr[ns + cnt]
        ne = e1 - e0
        assert ne <= enon
        base = tl * ecap
        # chunk 0: self loops of the tile's cnt nodes
        gs = core * nslot + tl * 128 + np.arange(cnt)
        hidx_orig[core, base:base + cnt] = gs
        hidx[core, base:base + cnt] = agrow(gs)
        dstloc[core, base:base + cnt] = np.arange(cnt, dtype=np.float32)
        # chunks 1..: real edges sorted by dst
        b1 = base + 128
        gs = pg[src_s[e0:e1]]
        hidx_orig[core, b1:b1 + ne] = gs
        hidx[core, b1:b1 + ne] = agrow(gs)
        dstloc[core, b1:b1 + ne] = (dst_s[e0:e1] - ns).astype(np.float32)
        waste_num += enon - ne

    def wrap_idx(a):
        # [S*128] -> [128, S*8]: idx i of gather g at [i%16, g*8 + i//16],
        # replicated across the 8 16-partition groups. One dma_gather per tile
        # uses a [128, cpt*8] slice.
        out = np.zeros((128, S * 8), np.int16)
        for g in range(S // cpt):  # per tile
            blk = a[g * ecap:(g + 1) * ecap].reshape(-1, 16)  # [cpt*8, 16]
            for rep in range(8):
                out[rep * 16:(rep + 1) * 16, g * cpt * 8:(g + 1) * cpt * 8] = blk.T
        return out

    meta = {
        "tiles": tiles, "pg": pg, "node_of_slot": node_of_slot,
        "hidx": np.stack([wrap_idx(hidx[c]) for c in range(N_CORES)]),
        "dstloc": np.stack([dstloc[c].reshape(S, 128).T for c in range(N_CORES)]),
        "hidx_flat": hidx_orig, "dstloc_flat": dstloc,
        "waste_frac": waste_num / (S * 128 * N_CORES),
    }
    return meta


# ------------------------------------------------------------- device program

def build_program(cfg):
    nc = bacc.Bacc("TRN2", target_bir_lowering=False, debug=False,
                   enable_asserts=False, num_devices=N_CORES,
                   dynamic_dma_scratch_size=16384)
    tpc, cpt, nslot = cfg.tpc, cfg.cpt, cfg.nslot
    S = tpc * cpt
    H, HD = cfg.heads, cfg.hid
    # Packed h-row layouts. Layers 1-2 (bf16-elem width 384 = 768 bytes):
    #   [8 x (64 fp8 h | fp8 1.0 marker) = 520B | 8 bf16 a_src | pad].
    # The 1.0 marker column folds the softmax denominator into the
    # aggregation matmul (numerator and denominator share one PE chain).
    # Layer 3 (bf16 width 128): [64 h | 1.0 marker | a_src | pad].
    HR = 384
    HR3 = 128
    ASOFF = 260          # bf16-elem offset of a_src in layers 1-2 rows

    def din(name, shape, dt):
        return nc.dram_tensor(name, list(shape), dt, kind="ExternalInput")

    xT = din("xT", [128, cfg.kc * nslot], BF16)
    Ws = [din(f"W{i+1}", [128, cfg.kc, w], BF16)
          for i, w in enumerate([512, 512, cfg.out_dim])]
    As = [din(f"As{i+1}", [128, w], BF16) for i, w in enumerate([512, 512, 64])]
    Ad = [din(f"Ad{i+1}", [128, w], BF16) for i, w in enumerate([512, 512, 64])]
    Bs = [din(f"b{i+1}", [128, w], F32) for i, w in enumerate([512, 512, 64])]
    hidx_t = din("hidx", [128, S * 8], I16)
    rs_t = din("rs", [128, S * 128], FP8)
    rt_t = din("rt", [128, S * 128], FP8)
    ident_t = din("ident", [128, 128], BF16)
    out_t = nc.dram_tensor("out", [nslot, cfg.out_dim], F32, kind="ExternalOutput")

    with tile.TileContext(nc) as tc:
        with tc.tile_pool(name="const", bufs=1) as cst, \
             tc.tile_pool(name="dram", bufs=1, space="DRAM") as dram, \
             tc.tile_pool(name="work", bufs=2) as wk, \
             tc.tile_pool(name="gath", bufs=4) as gp, \
             tc.tile_pool(name="ps", bufs=2, space="PSUM") as ps:

            # ---- persistent SBUF constants
            def load_const(t, shape, dt):
                s = cst.tile(shape, dt, name=t.name + "_sb")
                nc.sync.dma_start(s[:], t.ap())
                return s

            W_sb = [load_const(w, list(w.shape), BF16) for w in Ws]
            As_sb = [load_const(a, list(a.shape), BF16) for a in As]
            Ad_sb = [load_const(a, list(a.shape), BF16) for a in Ad]
            B_sb = [load_const(b, list(b.shape), F32) for b in Bs]
            hidx_sb = load_const(hidx_t, [128, S * 8], I16)
            ident_sb = load_const(ident_t, [128, 128], BF16)

            # input^T slab (lhsT source for dense matmuls), refreshed per layer
            inT = cst.tile([128, cfg.kc * nslot], BF16, name="inT")
            nc.sync.dma_start(inT[:], xT.ap())

            # a_dst values per tile (bf16, matmul rhs)
            advb = cst.tile([128, tpc, 8], BF16, name="advb")
            # layer-3 log_softmax staging (normalized outputs + exp-sums)
            onbuf = cst.tile([128, tpc * 64], F32, name="onbuf")
            zbuf = cst.tile([128, tpc], F32, name="zbuf")
            lzb = cst.tile([128, tpc], F32, name="lzb")

            h_owns = [dram.tile([nslot, HR if li < 2 else HR3], BF16,
                                name=f"h_own_{li}") for li in range(3)]
            h_alls = [dram.tile([N_CORES * nslot, HR if li < 2 else HR3], BF16,
                                name=f"h_all_{li}", addr_space="Shared")
                      for li in range(3)]

            rg = [list(range(N_CORES))]
            H1, H2 = cfg.h1, cfg.h2

            def phase_a_chunk(li, j):
                ow = 512 if li < 2 else cfg.out_dim
                nh = H if li < 2 else 1
                hw_ = ow // nh
                hrw = HR if li < 2 else HR3
                my_h_own = h_owns[li]
                hps = ps.tile([128, ow], F32, name="hps", tag="psA", bufs=4)
                for k in range(cfg.kc):
                    nc.tensor.matmul(
                        hps[:], lhsT=inT[:, k * nslot + j * 128:
                                         k * nslot + (j + 1) * 128],
                        rhs=W_sb[li][:, k, :],
                        start=(k == 0), stop=(k == cfg.kc - 1))
                hrow = wk.tile([128, hrw], BF16, name="hrow", tag="hrow")
                if li < 2:
                    # fp8 h strided into 65-wide head groups; marker col = 1.0
                    h8 = hrow[:, 0:ASOFF].bitcast(FP8).rearrange(
                        "p (h w) -> p h w", w=65)
                    nc.scalar.activation(
                        h8[:, :, 0:64],
                        hps[:].rearrange("p (h w) -> p h w", h=nh), AF.Copy)
                    nc.vector.memset(h8[:, :, 64:65], 1.0)
                    asoff = ASOFF
                else:
                    nc.scalar.activation(hrow[:, 0:ow], hps[:], AF.Copy)
                    nc.vector.memset(hrow[:, ow:ow + 1], 1.0)
                    asoff = ow + 1
                tmp = wk.tile([128, ow], BF16, name="atmp", tag="atmp")
                nc.vector.tensor_tensor(out=tmp[:], in0=hps[:],
                                        in1=As_sb[li][:, 0:ow], op=OP.mult)
                with nc.allow_low_precision(reason="bf16 attn logits"):
                    nc.vector.tensor_reduce(
                        out=hrow[:, asoff:asoff + nh],
                        in_=tmp[:].rearrange("p (h w) -> p h w", h=nh),
                        axis=mybir.AxisListType.X, op=OP.add)
                nc.vector.tensor_tensor(out=tmp[:], in0=hps[:],
                                        in1=Ad_sb[li][:, 0:ow], op=OP.mult)
                with nc.allow_low_precision(reason="bf16 attn logits"):
                    nc.vector.tensor_reduce(
                        out=advb[:, j, 0:nh],
                        in_=tmp[:].rearrange("p (h w) -> p h w", h=nh),
                        axis=mybir.AxisListType.X, op=OP.add)
                nc.scalar.dma_start(my_h_own[j * 128:(j + 1) * 128, :], hrow[:])

            def phase_b(li, half):
                if half == 0:
                    return
                nc.gpsimd.collective_compute(
                    "AllGather", OP.bypass, replica_groups=rg,
                    ins=[h_owns[li][:].opt()], outs=[h_alls[li][:].opt()])

            jhalf = tpc // 2 - 1
            for j in range(tpc):
                phase_a_chunk(0, j)
            phase_b(0, 1)

            for li in range(3):
                ow = 512 if li < 2 else cfg.out_dim       # h width this layer
                nh = H if li < 2 else 1                   # heads
                hw = HD if li < 2 else cfg.out_dim        # per-head width
                hrw = HR if li < 2 else HR3
                my_h_all = h_alls[li]
                mw = nh * (hw + 1)                        # msg width incl markers

                # ---------- phase C: per dst-tile edge processing
                GS = min(8, cpt)  # chunks per dma_gather (1024 descriptors max)
                assert cpt % GS == 0
                for t in range(tpc):
                    # selection matrices for this tile (graph-static, fp8)
                    rt = wk.tile([128, cpt, 128], FP8, name="rt", tag="rt", bufs=4)
                    nc.sync.dma_start(
                        rt[:].rearrange("p c d -> p (c d)"),
                        rt_t.ap()[:, t * cpt * 128:(t + 1) * cpt * 128])
                    Rs = wk.tile([128, cpt, 128], FP8, name="Rs", tag="Rs", bufs=4)
                    nc.sync.dma_start(
                        Rs[:].rearrange("p c d -> p (c d)"),
                        rs_t.ap()[:, t * cpt * 128:(t + 1) * cpt * 128])

                    hg = gp.tile([128, cpt, hrw], BF16, name="hg", tag="hg")
                    # chunk 0 = self loops: direct DMA from the local rows
                    nc.sync.dma_start(hg[:, 0, :],
                                      h_owns[li][t * 128:(t + 1) * 128, :])
                    # chunks 1.. : gathered edge rows
                    for g in range(1, cpt, GS):
                        gn = min(GS, cpt - g)
                        i0 = (t * cpt + g) * 8
                        nc.gpsimd.dma_gather(
                            out_ap=hg[:, g:g + gn, :], in_ap=my_h_all[:],
                            idxs_ap=hidx_sb[:, i0:i0 + gn * 8],
                            num_idxs=gn * 128, num_idxs_reg=gn * 128,
                            elem_size=hrw)

                    # Aggregation accumulators; bank 'a' is first reused for
                    # the per-edge a_dst broadcast (lifetimes don't overlap).
                    mh = mw // 2 if li < 2 else mw
                    pCa = ps.tile([128, 512], F32, name="pCa", tag="psCa")
                    pCb = ps.tile([128, 512], F32, name="pCb", tag="psCb") \
                        if li < 2 else None

                    # a_dst per edge via PE: adpe[e, c, h] = rt_c^T @ advb[t]
                    adpe = pCa[:, 0:cpt * nh].rearrange("p (c h) -> p c h", c=cpt)
                    for c in range(cpt):
                        nc.tensor.matmul(adpe[:, c, :],
                                         lhsT=rt[:, c, :],
                                         rhs=advb[:, t, 0:nh],
                                         start=True, stop=True)

                    # e = a_src[src] + a_dst[dst]; ex = exp(leaky_relu(e))
                    asoff = ASOFF if li < 2 else ow + 1
                    ee = wk.tile([128, cpt, nh], F32, name="ee", tag="ee")
                    nc.vector.tensor_tensor(
                        out=ee[:], in0=hg[:, :, asoff:asoff + nh],
                        in1=adpe, op=OP.add)
                    nc.vector.scalar_tensor_tensor(
                        out=ee[:], in0=ee[:], scalar=NEG_SLOPE, in1=ee[:],
                        op0=OP.mult, op1=OP.max)
                    exb = wk.tile([128, cpt, nh], BF16, name="exb", tag="exb")
                    nc.scalar.activation(exb[:], ee[:], AF.Exp)

                    # msg strip incl denominator markers:
                    # ms[e, c, h, 0:hw] = h[e] * ex[e,h]; ms[e, c, h, hw] = ex
                    if li < 2:
                        hgsl = hg[:, :, 0:ASOFF].bitcast(FP8)
                    else:
                        hgsl = hg[:, :, 0:mw]
                    ms = wk.tile([128, cpt, mw], BF16, name="ms", tag="ms")
                    nc.vector.scalar_tensor_tensor(
                        out=ms[:].rearrange("p c (h w) -> p c h w", h=nh),
                        in0=hgsl.rearrange("p c (h w) -> p c h w", h=nh),
                        scalar=1.0,
                        in1=exb[:].rearrange("p c (h o) -> p c h o", o=1)
                            .to_broadcast([128, cpt, nh, hw + 1]),
                        op0=OP.mult, op1=OP.mult)

                    # aggregation: two accumulation chains (each <=1 PSUM bank)
                    chains = [(pCa, 0)]
                    if li < 2:
                        chains.append((pCb, mh))
                    for c in range(cpt):
                        for pc_, off in chains:
                            nc.tensor.matmul(pc_[:, 0:mh], lhsT=Rs[:, c, :],
                                             rhs=ms[:, c, off:off + mh],
                                             start=(c == 0), stop=(c == cpt - 1))

                    # normalize: on[d, h, :] = num / (den + eps), + bias
                    rec = wk.tile([128, nh], F32, name="rec", tag="rec")
                    nhh = nh // len(chains)
                    for i, (pc_, off) in enumerate(chains):
                        nc.vector.tensor_scalar(
                            out=rec[:, i * nhh:(i + 1) * nhh],
                            in0=pc_[:, 0:mh].rearrange("p (h w) -> p h w", w=hw + 1)
                                [:, :, hw:hw + 1].rearrange("p h o -> p (h o)"),
                            scalar1=1e-16, scalar2=None, op0=OP.add)
                    nc.vector.reciprocal(rec[:], rec[:])
                    on = wk.tile([128, ow], F32, name="on", tag="on") \
                        if li < 2 else onbuf[:, t * 64:(t + 1) * 64]
                    for i, (pc_, off) in enumerate(chains):
                        nc.vector.tensor_tensor(
                            out=on[:, i * nhh * hw:(i + 1) * nhh * hw].rearrange(
                                "p (h w) -> p h w", h=nhh),
                            in0=pc_[:, 0:mh].rearrange("p (h w) -> p h w", w=hw + 1)
                                [:, :, 0:hw],
                            in1=rec[:, i * nhh:(i + 1) * nhh].rearrange(
                                "p (h o) -> p h o", o=1)
                                .to_broadcast([128, nhh, hw]),
                            op=OP.mult)
                    if cfg.has_bias:
                        nc.vector.tensor_tensor(out=on[:], in0=on[:],
                                                in1=B_sb[li][:, 0:ow], op=OP.add)

                    if li < 2:
                        # ELU: elu(x) = max(x,0) + (exp(-relu(-x)) - 1)
                        rn = wk.tile([128, ow], F32, name="rn", tag="rn")
                        nc.scalar.activation(rn[:], on[:], AF.Relu, scale=-1.0)
                        nc.scalar.activation(rn[:], rn[:], AF.Exp, scale=-1.0)
                        nc.scalar.activation(rn[:], rn[:], AF.Copy, bias=-1.0)
                        o2b = wk.tile([128, ow], BF16, name="o2b", tag="o2b")
                        nc.vector.scalar_tensor_tensor(
                            out=o2b[:], in0=on[:], scalar=0.0, in1=rn[:],
                            op0=OP.max, op1=OP.add)
                        for k in range(cfg.kc):
                            tp = ps.tile([128, 128], BF16, name="tp", tag="psA",
                                         bufs=4)
                            nc.tensor.transpose(tp[:], o2b[:, k * 128:(k + 1) * 128],
                                                ident_sb[:])
                            nc.scalar.activation(
                                inT[:, k * nslot + t * 128:k * nslot + (t + 1) * 128],
                                tp[:], AF.Copy)
                    else:
                        # exp-sum for the deferred log_softmax (values are
                        # small, so no max subtraction is needed)
                        pe_ = wk.tile([128, ow], F32, name="pe_", tag="pe_")
                        nc.scalar.activation(pe_[:], on[:], AF.Exp,
                                             accum_out=zbuf[:, t:t + 1])
                    if li < 2:
                        phase_a_chunk(li + 1, t)
                        if t == jhalf:
                            phase_b(li + 1, 0)
                if li < 2:
                    phase_b(li + 1, 1)

            # deferred log_softmax epilogue: one Ln for all tiles, then
            # fin = on - log(z) per tile
            nc.scalar.activation(lzb[:], zbuf[:], AF.Ln)
            for t in range(tpc):
                fin = wk.tile([128, 64], F32, name="fin", tag="fin")
                nc.vector.scalar_tensor_tensor(
                    out=fin[:], in0=onbuf[:, t * 64:(t + 1) * 64], scalar=1.0,
                    in1=lzb[:, t:t + 1].to_broadcast([128, 64]),
                    op0=OP.mult, op1=OP.subtract)
                nc.scalar.dma_start(out_t.ap()[t * 128:(t + 1) * 128, :], fin[:])

    nc.compile()
    return nc


# ------------------------------------------------------------------ inputs

def make_in_maps(cfg, meta, x, Ws, As_, Ad_, Bs_):
    bf = ml_dtypes.bfloat16
    f8 = ml_dtypes.float8_e4m3
    n, nslot, S = cfg.n, cfg.nslot, cfg.tpc * cfg.cpt
    node_of_slot = meta["node_of_slot"]

    # xT: [128, kc*nslot] per core
    xpad = np.zeros((N_CORES * nslot, cfg.in_dim), np.float32)
    valid = node_of_slot >= 0
    xpad[valid] = x[node_of_slot[valid]]

    ident = np.eye(128, dtype=np.float32).astype(bf)

    def bcast(v, w):
        out = np.zeros((128, w), np.float32)
        out[:, :v.size] = np.tile(v.reshape(1, -1), (128, 1))
        return out

    common = {}
    for i, W in enumerate(Ws):
        kc = cfg.kc
        common[f"W{i+1}"] = W.reshape(kc, 128, W.shape[1]).transpose(1, 0, 2).astype(bf)
    for i, (a_s, a_d) in enumerate(zip(As_, Ad_)):
        w = 512 if i < 2 else 64
        common[f"As{i+1}"] = bcast(a_s.reshape(-1), w).astype(bf)
        common[f"Ad{i+1}"] = bcast(a_d.reshape(-1), w).astype(bf)
    for i, b in enumerate(Bs_):
        w = 512 if i < 2 else 64
        common[f"b{i+1}"] = bcast(b.reshape(-1), w)
    common["ident"] = ident

    in_maps = []
    for c in range(N_CORES):
        xc = xpad[c * nslot:(c + 1) * nslot]                 # [nslot, in_dim]
        # want [128, kc*nslot] with [p, k*nslot+s] = x[s, k*128+p]
        xTl = np.zeros((128, cfg.kc * nslot), np.float32)
        for k in range(cfg.kc):
            xTl[:, k * nslot:(k + 1) * nslot] = xc[:, k * 128:(k + 1) * 128].T
        m = dict(common)
        m["xT"] = xTl.astype(bf)
        m["hidx"] = meta["hidx"][c]
        dl = meta["dstloc"][c]                       # [128, S]
        dgrid = np.arange(128, dtype=np.float32)
        m["rs"] = (dl[:, :, None] == dgrid[None, None, :]).reshape(
            128, -1).astype(f8)                      # [p,(s d)]
        m["rt"] = (dl.T[None, :, :] == dgrid[:, None, None]).reshape(
            128, -1).astype(f8)                      # [d-part,(s e)]

        in_maps.append(m)
    return in_maps


# ------------------------------------------------------------------- kernel

@functools.lru_cache(maxsize=1)
def _get_program_and_meta_cached(edge_key):
    cfg, edge_index = _PENDING[edge_key]
    meta = prep(cfg, edge_index)
    nc = build_program(cfg)
    return cfg, meta, nc


_PENDING = {}


def _program_for(edge_index, has_bias=True):
    key = (hash(edge_index.tobytes()), bool(has_bias))
    if key not in _PENDING:
        cfg = Cfg(n=20000, e=edge_index.shape[1], has_bias=has_bias)
        _PENDING[key] = (cfg, np.asarray(edge_index))
    return _get_program_and_meta_cached(key)


def _setup_trace_shims():
    """Register the NTFF profile hook the container's antenv stub lacks, and
    neuter the S3 artifact upload. Only needed for trace=True runs."""
    import types
    import antenv
    if "antenv.axon_hooks" not in sys.modules:
        mod = types.ModuleType("antenv.axon_hooks")
        mod._hook = None

        def set_axon_ntff_profile_hook(h):
            mod._hook = h

        def get_axon_ntff_profile_hook():
            return mod._hook

        mod.set_axon_ntff_profile_hook = set_axon_ntff_profile_hook
        mod.get_axon_ntff_profile_hook = get_axon_ntff_profile_hook
        sys.modules["antenv.axon_hooks"] = mod
        antenv.axon_hooks = mod
        try:
            from trn_agent_boot.trn_boot import _ntff_profile_via_ctypes
            set_axon_ntff_profile_hook(
                _ntff_profile_via_ctypes("/opt/axon/libaxon_pjrt.so"))
        except Exception as ex:  # pragma: no cover
            print(f"ntff hook setup failed: {ex}", file=sys.stderr)
    bass_utils.upload_artifacts = lambda tmpdir: tmpdir


def run(inputs, trace=False, trace_kwargs=None):
    if trace:
        try:
            _setup_trace_shims()
        except Exception as ex:
            print(f"trace shims failed ({ex}); running untraced", file=sys.stderr)
            trace = False
    x = np.asarray(inputs["x"], np.float32)
    edge_index = np.asarray(inputs["edge_index"])
    has_bias = any(np.any(np.asarray(inputs[f"b{i+1}"]) != 0) for i in range(3))
    cfg, meta, nc = _program_for(edge_index, has_bias)
    in_maps = make_in_maps(
        cfg, meta, x,
        [np.asarray(inputs[f"W{i+1}"], np.float32) for i in range(3)],
        [np.asarray(inputs[f"as{i+1}"], np.float32) for i in range(3)],
        [np.asarray(inputs[f"ad{i+1}"], np.float32) for i in range(3)],
        [np.asarray(inputs[f"b{i+1}"], np.float32) for i in range(3)],
    )
    res = bass_utils.run_bass_kernel_spmd(
        nc, in_maps, core_ids=list(range(N_CORES)), trace=trace,
        **(trace_kwargs or {}))
    node_of_slot = meta["node_of_slot"]
    out = np.zeros((cfg.n, cfg.out_dim), np.float32)
    for c in range(N_CORES):
        o = np.asarray(res.results[c]["out"], np.float32)
        sl = node_of_slot[c * cfg.nslot:(c + 1) * cfg.nslot]
        v = sl >= 0
        out[sl[v]] = o[v]
    return out, res


def kernel(**inputs) -> np.ndarray:
    out, _ = run(inputs)
    return out
